# revision 7
# baseline (speedup 1.0000x reference)
"""Trainium2 Bass kernel for Conv2dWeightModulate (StyleGAN2-style modulated conv).

Math restructure: the per-sample modulated conv
    out[b] = conv(conv_w * c * style[b,cin] * sigma_inv[b,cout], x_pad[b])
is rewritten as
    out[b,cout] = sigma_inv[b,cout] * conv(conv_w, (x[b] * c*style[b,cin])_pad)
so the conv weights are sample-independent (loaded to SBUF once) and the
per-sample modulation becomes a per-input-channel scale of x plus a
per-output-channel scale of the result. sigma has the closed form
    sigma^2[b,cout] = c^2 * sum_cin style[b,cin]^2 * sum_k conv_w[cout,cin,k]^2
computed on host (tiny [B,CIN] x [CIN,COUT] product), as is the 3-layer
mapping network producing style (all [16,512]-sized, <0.01% of FLOPs).

Device: data-parallel over batch, 2 samples per core on 8 cores. Per
(sample, half-image, cout-block): 9 taps x 4 cin-blocks = 36 accumulating
128x128 @ 128x512 matmuls per PSUM bank, 4 banks (8 output rows each) per
group, float32r matmul mode (fp32 data, full PE rate at N=512; the PE
rounds operands at ingest, ~13 effective mantissa bits). Replicate-padding
is built in SBUF with small DVE copies; the per-cout sigma_inv scale rides
the mandatory PSUM->SBUF eviction on DVE.
"""

import numpy as np
from contextlib import ExitStack

import concourse.bass as bass
import concourse.tile as tile
from concourse import bacc, mybir
from concourse import bass_utils

B, CIN, COUT, KS, H, W, DLAT = 16, 512, 512, 3, 64, 64, 512
EPS = 1e-8
N_CORES = 8
SPC = B // N_CORES          # samples per core
NCB = CIN // 128            # cin blocks
NOB = COUT // 128           # cout blocks
HALF = H // 2               # rows per half-image
RPC = 8                     # output rows per PSUM chunk (8*64 = 512 = 1 bank)
RC = HALF // RPC            # row chunks per half
PADW = W + 2
PADR = HALF + 2
NTAP = KS * KS

_cache = {}


def _build():
    if "nc" in _cache:
        return _cache["nc"]
    f32 = mybir.dt.float32
    f32r = mybir.dt.float32r
    nc = bacc.Bacc("TRN2", target_bir_lowering=False, debug=False,
                   num_devices=N_CORES)
    # x is pre-scaled by c*style[b,cin] on host; f32r dtype feeds the PE's
    # fp32r matmul path directly (PE rounds at ingest).
    x_d = nc.dram_tensor("x", [SPC, CIN, H, W], f32r, kind="ExternalInput").ap()
    wt_d = nc.dram_tensor("wt", [NCB, 128, NTAP, COUT], f32r,
                          kind="ExternalInput").ap()
    sig_d = nc.dram_tensor("sig", [128, SPC, NOB], f32,
                           kind="ExternalInput").ap()
    out_d = nc.dram_tensor("out", [SPC, COUT, H * W], f32,
                           kind="ExternalOutput").ap()

    with tile.TileContext(nc) as tc, ExitStack() as ctx:
        cpool = ctx.enter_context(tc.tile_pool(name="const", bufs=1))
        xpool = ctx.enter_context(tc.tile_pool(name="x", bufs=2))
        opool = ctx.enter_context(tc.tile_pool(name="o", bufs=6))
        pspool = ctx.enter_context(tc.tile_pool(name="ps", bufs=8, space="PSUM"))

        wt_sb = cpool.tile([128, NCB, NTAP, COUT], f32r)
        for cb in range(NCB):
            nc.sync.dma_start(wt_sb[:, cb], wt_d[cb])
        sig_sb = cpool.tile([128, SPC, NOB], f32)
        nc.sync.dma_start(sig_sb[:], sig_d[:])

        for s in range(SPC):
            for hf in range(2):
                # x_t row r holds padded row 32*hf + r (padded row p maps to
                # original row clamp(p-1, 0, H-1)); col q holds padded col q.
                x_t = xpool.tile([128, NCB, PADR, PADW], f32r)
                r0, rsb0 = (0, 1) if hf == 0 else (HALF - 1, 0)
                nrows = HALF + 1  # 33 original rows cover this half
                for cb in range(NCB):
                    nc.sync.dma_start(
                        x_t[:, cb, rsb0:rsb0 + nrows, 1:W + 1],
                        x_d[s, cb * 128:(cb + 1) * 128, r0:r0 + nrows, :])
                    # replicate-pad columns, then the one missing row
                    nc.vector.tensor_copy(
                        x_t[:, cb, rsb0:rsb0 + nrows, 0],
                        x_t[:, cb, rsb0:rsb0 + nrows, 1])
                    nc.vector.tensor_copy(
                        x_t[:, cb, rsb0:rsb0 + nrows, W + 1],
                        x_t[:, cb, rsb0:rsb0 + nrows, W])
                    if hf == 0:
                        nc.vector.tensor_copy(x_t[:, cb, 0, :], x_t[:, cb, 1, :])
                    else:
                        nc.vector.tensor_copy(x_t[:, cb, PADR - 1, :],
                                              x_t[:, cb, PADR - 2, :])

                for ob in range(NOB):
                    pts = [pspool.tile([128, RPC * W], f32, name="ps", tag="ps")
                           for _ in range(RC)]
                    for tap in range(NTAP):
                        kh, kw = divmod(tap, KS)
                        for cb in range(NCB):
                            lhsT = wt_sb[:, cb, tap, ob * 128:(ob + 1) * 128]
                            for rc in range(RC):
                                rhs = x_t[:, cb, rc * RPC + kh:rc * RPC + kh + RPC,
                                          kw:kw + W]
                                nc.tensor.matmul(
                                    pts[rc][:], lhsT, rhs,
                                    start=(tap == 0 and cb == 0),
                                    stop=(tap == NTAP - 1 and cb == NCB - 1))
                    for rc in range(RC):
                        o = opool.tile([128, RPC * W], f32)
                        nc.vector.tensor_scalar_mul(o[:], pts[rc][:],
                                                    sig_sb[:, s, ob:ob + 1])
                        row0 = hf * HALF + rc * RPC
                        nc.sync.dma_start(
                            out_d[s, ob * 128:(ob + 1) * 128,
                                  row0 * W:(row0 + RPC) * W], o[:])
    nc.compile()
    _cache["nc"] = nc
    return nc


def _prelu(z, a):
    return np.where(z >= 0, z, a * z)


def _prepare(inputs):
    x = np.asarray(inputs["x"], dtype=np.float32)
    s = np.asarray(inputs["s"], dtype=np.float32)
    map_w0 = np.asarray(inputs["map_w0"], dtype=np.float32)
    map_b0 = np.asarray(inputs["map_b0"], dtype=np.float32)
    a0 = np.asarray(inputs["prelu_a0"], dtype=np.float32)
    map_w1 = np.asarray(inputs["map_w1"], dtype=np.float32)
    map_b1 = np.asarray(inputs["map_b1"], dtype=np.float32)
    a1 = np.asarray(inputs["prelu_a1"], dtype=np.float32)
    style_w = np.asarray(inputs["style_w"], dtype=np.float32)
    style_b = np.asarray(inputs["style_b"], dtype=np.float32)
    conv_w = np.asarray(inputs["conv_w"], dtype=np.float32)

    c_lin = np.float32(1.0 / np.sqrt(DLAT))
    z = _prelu(s @ (map_w0 * c_lin).T + map_b0, a0)
    z = _prelu(z @ (map_w1 * c_lin).T + map_b1, a1)
    style = z @ (style_w * c_lin).T + style_b          # [B, CIN]

    c_conv = 1.0 / np.sqrt(CIN * KS * KS)
    w2 = ((conv_w.astype(np.float64) * c_conv) ** 2).sum(axis=(2, 3))  # [COUT, CIN]
    sig2 = (style.astype(np.float64) ** 2) @ w2.T                      # [B, COUT]
    sig_inv = (1.0 / np.sqrt(sig2 + EPS)).astype(np.float32)
    msc = (style * np.float32(c_conv)).astype(np.float32)              # [B, CIN]

    # fold the per-cin style scale into x on host (fp32, same as reference's
    # fp32 weight modulation up to reassociation)
    x_scaled = x * msc[:, :, None, None]

    # [NCB, 128, NTAP, COUT]
    wt_host = np.ascontiguousarray(
        conv_w.reshape(COUT, NCB, 128, NTAP).transpose(1, 2, 3, 0))

    sig_r = sig_inv.reshape(B, NOB, 128)
    in_maps = []
    for c in range(N_CORES):
        sl = slice(c * SPC, (c + 1) * SPC)
        in_maps.append({
            "x": np.ascontiguousarray(x_scaled[sl]),
            "wt": wt_host,
            "sig": np.ascontiguousarray(sig_r[sl].transpose(2, 0, 1)),
        })
    return in_maps


def run(inputs, **spmd_kwargs):
    nc = _build()
    in_maps = _prepare(inputs)
    res = bass_utils.run_bass_kernel_spmd(
        nc, in_maps, core_ids=list(range(N_CORES)), **spmd_kwargs)
    out = np.concatenate(
        [res.results[c]["out"].reshape(SPC, COUT, H, W)
         for c in range(N_CORES)], axis=0)
    return out, res


def kernel(**inputs) -> np.ndarray:
    out, _ = run(inputs)
    return out


# revision 8
# speedup vs baseline: 1.1364x; 1.1364x over previous
"""Trainium2 Bass kernel for Conv2dWeightModulate (StyleGAN2-style modulated conv).

Math restructure: the per-sample modulated conv
    out[b] = conv(conv_w * c * style[b,cin] * sigma_inv[b,cout], x_pad[b])
is rewritten as
    out[b,cout] = sigma_inv[b,cout] * conv(conv_w, (x[b] * c*style[b,cin])_pad)
so the conv weights are sample-independent (loaded to SBUF once) and the
per-sample modulation becomes a per-input-channel scale of x plus a
per-output-channel scale of the result. sigma has the closed form
    sigma^2[b,cout] = c^2 * sum_cin style[b,cin]^2 * sum_k conv_w[cout,cin,k]^2
computed on host (tiny [B,CIN] x [CIN,COUT] product), as is the 3-layer
mapping network producing style (all [16,512]-sized, <0.01% of FLOPs).

Device: data-parallel over batch, 2 samples per core on 8 cores. Per
(sample, half-image, cout-block): 9 taps x 4 cin-blocks = 36 accumulating
128x128 @ 128x512 matmuls per PSUM bank, 4 banks (8 output rows each) per
group. Inputs are fp16 (PE runs 1 col/cycle; fp32 accumulate in PSUM); a
per-sample power-of-2 prescale keeps the scaled x in fp16's normal range
and is undone exactly in the fp32 output scale. Replicate-padding is built
in SBUF with small DVE copies; the per-cout sigma_inv scale rides the
mandatory PSUM->SBUF eviction on DVE.
"""

import numpy as np
from contextlib import ExitStack

import concourse.bass as bass
import concourse.tile as tile
from concourse import bacc, mybir
from concourse import bass_utils

B, CIN, COUT, KS, H, W, DLAT = 16, 512, 512, 3, 64, 64, 512
EPS = 1e-8
N_CORES = 8
SPC = B // N_CORES          # samples per core
NCB = CIN // 128            # cin blocks
NOB = COUT // 128           # cout blocks
HALF = H // 2               # rows per half-image
RPC = 8                     # output rows per PSUM chunk (8*64 = 512 = 1 bank)
RC = HALF // RPC            # row chunks per half
PADW = W + 2
PADR = HALF + 2
NTAP = KS * KS

_cache = {}


def _build():
    if "nc" in _cache:
        return _cache["nc"]
    f32 = mybir.dt.float32
    f16 = mybir.dt.float16
    nc = bacc.Bacc("TRN2", target_bir_lowering=False, debug=False,
                   num_devices=N_CORES)
    # x is pre-scaled by c*style[b,cin] (and a per-sample 2^k normalizer)
    # and cast to fp16 on host; weights are conv_w cast to fp16.
    x_d = nc.dram_tensor("x", [SPC, CIN, H, W], f16, kind="ExternalInput").ap()
    wt_d = nc.dram_tensor("wt", [NCB, 128, NTAP, COUT], f16,
                          kind="ExternalInput").ap()
    sig_d = nc.dram_tensor("sig", [128, SPC, NOB], f32,
                           kind="ExternalInput").ap()
    out_d = nc.dram_tensor("out", [SPC, COUT, H * W], f32,
                           kind="ExternalOutput").ap()

    with tile.TileContext(nc) as tc, ExitStack() as ctx:
        cpool = ctx.enter_context(tc.tile_pool(name="const", bufs=1))
        xpool = ctx.enter_context(tc.tile_pool(name="x", bufs=3))
        opool = ctx.enter_context(tc.tile_pool(name="o", bufs=6))
        pspool = ctx.enter_context(tc.tile_pool(name="ps", bufs=8, space="PSUM"))

        wt_sb = cpool.tile([128, NCB, NTAP, COUT], f16)
        sig_sb = cpool.tile([128, SPC, NOB], f32)

        for s in range(SPC):
            for hf in range(2):
                # x_t row r holds padded row 32*hf + r (padded row p maps to
                # original row clamp(p-1, 0, H-1)); col q holds padded col q.
                x_t = xpool.tile([128, NCB, PADR, PADW], f16)
                r0, rsb0 = (0, 1) if hf == 0 else (HALF - 1, 0)
                nrows = HALF + 1  # 33 original rows cover this half
                for cb in range(NCB):
                    if s == 0 and hf == 0:
                        # interleave weight loads with the first x loads so
                        # cin-block cb's matmuls can start as soon as its own
                        # slices have landed
                        nc.sync.dma_start(wt_sb[:, cb], wt_d[cb])
                    nc.sync.dma_start(
                        x_t[:, cb, rsb0:rsb0 + nrows, 1:W + 1],
                        x_d[s, cb * 128:(cb + 1) * 128, r0:r0 + nrows, :])
                    # replicate-pad columns, then the one missing row
                    nc.vector.tensor_copy(
                        x_t[:, cb, rsb0:rsb0 + nrows, 0],
                        x_t[:, cb, rsb0:rsb0 + nrows, 1])
                    nc.vector.tensor_copy(
                        x_t[:, cb, rsb0:rsb0 + nrows, W + 1],
                        x_t[:, cb, rsb0:rsb0 + nrows, W])
                    if hf == 0:
                        nc.vector.tensor_copy(x_t[:, cb, 0, :], x_t[:, cb, 1, :])
                    else:
                        nc.vector.tensor_copy(x_t[:, cb, PADR - 1, :],
                                              x_t[:, cb, PADR - 2, :])
                if s == 0 and hf == 0:
                    nc.sync.dma_start(sig_sb[:], sig_d[:])

                for ob in range(NOB):
                    pts = [pspool.tile([128, RPC * W], f32, name="ps", tag="ps")
                           for _ in range(RC)]
                    # cin-block-major accumulation: the first 9*RC matmuls only
                    # need x/wt of cin-block 0, overlapping later DMAs
                    for cb in range(NCB):
                        for tap in range(NTAP):
                            kh, kw = divmod(tap, KS)
                            lhsT = wt_sb[:, cb, tap, ob * 128:(ob + 1) * 128]
                            for rc in range(RC):
                                rhs = x_t[:, cb, rc * RPC + kh:rc * RPC + kh + RPC,
                                          kw:kw + W]
                                nc.tensor.matmul(
                                    pts[rc][:], lhsT, rhs,
                                    start=(cb == 0 and tap == 0),
                                    stop=(cb == NCB - 1 and tap == NTAP - 1))
                    for rc in range(RC):
                        o = opool.tile([128, RPC * W], f32)
                        nc.vector.tensor_scalar_mul(o[:], pts[rc][:],
                                                    sig_sb[:, s, ob:ob + 1])
                        row0 = hf * HALF + rc * RPC
                        nc.sync.dma_start(
                            out_d[s, ob * 128:(ob + 1) * 128,
                                  row0 * W:(row0 + RPC) * W], o[:])
    nc.compile()
    _cache["nc"] = nc
    return nc


def _prelu(z, a):
    return np.where(z >= 0, z, a * z)


def _prepare(inputs):
    x = np.asarray(inputs["x"], dtype=np.float32)
    s = np.asarray(inputs["s"], dtype=np.float32)
    map_w0 = np.asarray(inputs["map_w0"], dtype=np.float32)
    map_b0 = np.asarray(inputs["map_b0"], dtype=np.float32)
    a0 = np.asarray(inputs["prelu_a0"], dtype=np.float32)
    map_w1 = np.asarray(inputs["map_w1"], dtype=np.float32)
    map_b1 = np.asarray(inputs["map_b1"], dtype=np.float32)
    a1 = np.asarray(inputs["prelu_a1"], dtype=np.float32)
    style_w = np.asarray(inputs["style_w"], dtype=np.float32)
    style_b = np.asarray(inputs["style_b"], dtype=np.float32)
    conv_w = np.asarray(inputs["conv_w"], dtype=np.float32)

    c_lin = np.float32(1.0 / np.sqrt(DLAT))
    z = _prelu(s @ (map_w0 * c_lin).T + map_b0, a0)
    z = _prelu(z @ (map_w1 * c_lin).T + map_b1, a1)
    style = z @ (style_w * c_lin).T + style_b          # [B, CIN]

    c_conv = 1.0 / np.sqrt(CIN * KS * KS)
    w2 = ((conv_w.astype(np.float64) * c_conv) ** 2).sum(axis=(2, 3))  # [COUT, CIN]
    sig2 = (style.astype(np.float64) ** 2) @ w2.T                      # [B, COUT]
    sig_inv = (1.0 / np.sqrt(sig2 + EPS)).astype(np.float32)
    msc = (style * np.float32(c_conv)).astype(np.float32)              # [B, CIN]

    # per-sample power-of-2 normalizer keeps msc*x in fp16's normal range;
    # undone exactly in the fp32 output scale
    rms = np.sqrt(np.mean((msc.astype(np.float64)) ** 2, axis=1)) + 1e-30
    k = np.clip(np.round(-np.log2(rms)), -20, 40).astype(np.int32)     # [B]
    pw = np.exp2(k.astype(np.float32))                                  # 2^k
    msc_n = msc * pw[:, None]
    sig_n = sig_inv / pw[:, None]

    # fold the per-cin style scale into x on host, cast fp16
    x_scaled = (x * msc_n[:, :, None, None]).astype(np.float16)

    # [NCB, 128, NTAP, COUT] fp16
    wt_host = np.ascontiguousarray(
        conv_w.reshape(COUT, NCB, 128, NTAP).transpose(1, 2, 3, 0)
    ).astype(np.float16)

    sig_r = sig_n.reshape(B, NOB, 128)
    in_maps = []
    for c in range(N_CORES):
        sl = slice(c * SPC, (c + 1) * SPC)
        in_maps.append({
            "x": np.ascontiguousarray(x_scaled[sl]),
            "wt": wt_host,
            "sig": np.ascontiguousarray(sig_r[sl].transpose(2, 0, 1)),
        })
    return in_maps


def run(inputs, **spmd_kwargs):
    nc = _build()
    in_maps = _prepare(inputs)
    res = bass_utils.run_bass_kernel_spmd(
        nc, in_maps, core_ids=list(range(N_CORES)), **spmd_kwargs)
    out = np.concatenate(
        [res.results[c]["out"].reshape(SPC, COUT, H, W)
         for c in range(N_CORES)], axis=0)
    return out, res


def kernel(**inputs) -> np.ndarray:
    out, _ = run(inputs)
    return out


# revision 10
# speedup vs baseline: 1.5950x; 1.4035x over previous
"""Trainium2 Bass kernel for Conv2dWeightModulate (StyleGAN2-style modulated conv).

Math restructure 1 (modulation): the per-sample modulated conv
    out[b] = conv(conv_w * c * style[b,cin] * sigma_inv[b,cout], x_pad[b])
is rewritten as
    out[b,cout] = sigma_inv[b,cout] * conv(conv_w, (x[b] * c*style[b,cin])_pad)
so the conv weights are sample-independent (resident in SBUF) and the
per-sample modulation becomes a per-input-channel scale of x plus a
per-output-channel scale of the result. sigma has the closed form
    sigma^2[b,cout] = c^2 * sum_cin style[b,cin]^2 * sum_k conv_w[cout,cin,k]^2
computed on host (tiny [B,CIN] x [CIN,COUT] product), as is the 3-layer
mapping network producing style (all [16,512]-sized, <0.01% of FLOPs).

Math restructure 2 (Winograd F(2,3) along the height axis): each pair of
output rows (2p, 2p+1) is computed from 4 transformed input rows
    v0 = d[2p]-d[2p+2], v1 = d[2p+1]+d[2p+2],
    v2 = d[2p+2]-d[2p+1], v3 = d[2p+1]-d[2p+3]
with host-transformed weights U = G @ w over the kh axis
    (G = [[1,0,0],[.5,.5,.5],[.5,-.5,.5],[0,0,1]])
and output rows  out[2p] = M0+M1+M2,  out[2p+1] = M1-M2-M3  where
    M[pos] = sum_{cin,kw} U[pos,kw] * v[pos]  (shifted by kw).
This cuts tensor-engine MACs by 1.5x (12 accumulation steps per output
tile instead of 18 row-equivalents); the width axis stays direct (3 taps
against a replicate-padded 66-wide SBUF image).

Device: data-parallel over batch, 2 samples per core on 8 cores, fp16
operands (PE at 1 col/cycle, fp32 PSUM accumulate). Per (sample, half,
cout-block, row-chunk): 4 PSUM banks hold M[0..3] for 8 row-pairs x 64
cols; 48 accumulating 128x128 @ 128x512 matmuls fill them; ScalarE applies
sigma_inv during PSUM->SBUF eviction and VectorE forms the two output-row
combinations in fp32. A per-sample power-of-2 prescale keeps scaled x in
fp16's normal range and is undone exactly in the fp32 output scale.
"""

import numpy as np
from contextlib import ExitStack

import concourse.bass as bass
import concourse.tile as tile
from concourse import bacc, mybir
from concourse import bass_utils

B, CIN, COUT, KS, H, W, DLAT = 16, 512, 512, 3, 64, 64, 512
EPS = 1e-8
N_CORES = 8
SPC = B // N_CORES          # samples per core
NCB = CIN // 128            # cin blocks
NOB = COUT // 128           # cout blocks
HALF = H // 2               # rows per half-image
NPAIR = HALF // 2           # winograd row-pairs per half (16)
PPC = 8                     # row-pairs per PSUM chunk (8 pairs * 64 = 512)
NCH = NPAIR // PPC          # chunks per half (2)
PADW = W + 2
PADR = HALF + 2
NPOS = 4                    # winograd positions
_cache = {}


def _build():
    if "nc" in _cache:
        return _cache["nc"]
    f32 = mybir.dt.float32
    f16 = mybir.dt.float16
    nc = bacc.Bacc("TRN2", target_bir_lowering=False, debug=False,
                   num_devices=N_CORES)
    x_d = nc.dram_tensor("x", [SPC, CIN, H, W], f16, kind="ExternalInput").ap()
    # U[cb, p, pos, kw, cout]
    wt_d = nc.dram_tensor("wt", [NCB, 128, NPOS, KS, COUT], f16,
                          kind="ExternalInput").ap()
    sig_d = nc.dram_tensor("sig", [128, SPC, NOB], f32,
                           kind="ExternalInput").ap()
    out_d = nc.dram_tensor("out", [SPC, COUT, H * W], f32,
                           kind="ExternalOutput").ap()

    with tile.TileContext(nc) as tc, ExitStack() as ctx:
        cpool = ctx.enter_context(tc.tile_pool(name="const", bufs=1))
        xpool = ctx.enter_context(tc.tile_pool(name="x", bufs=1))
        vpool = ctx.enter_context(tc.tile_pool(name="v", bufs=2))
        smpool = ctx.enter_context(tc.tile_pool(name="sm", bufs=8))
        opool = ctx.enter_context(tc.tile_pool(name="o", bufs=8))
        pspool = ctx.enter_context(tc.tile_pool(name="ps", bufs=8, space="PSUM"))

        wt_sb = cpool.tile([128, NCB, NPOS, KS, COUT], f16)
        sig_sb = cpool.tile([128, SPC, NOB], f32)

        for s in range(SPC):
            for hf in range(2):
                # x_t row r holds padded row 32*hf + r (padded row p maps to
                # original row clamp(p-1, 0, H-1)); col q holds padded col q.
                x_t = xpool.tile([128, NCB, PADR, PADW], f16)
                v_t = vpool.tile([128, NCB, NPOS, NPAIR, PADW], f16)
                r0, rsb0 = (0, 1) if hf == 0 else (HALF - 1, 0)
                nrows = HALF + 1  # 33 original rows cover this half
                for cb in range(NCB):
                    if s == 0 and hf == 0:
                        # interleave weight loads with the first x loads so
                        # cin-block cb's matmuls can start as soon as its own
                        # slices have landed
                        nc.sync.dma_start(wt_sb[:, cb], wt_d[cb])
                    nc.sync.dma_start(
                        x_t[:, cb, rsb0:rsb0 + nrows, 1:W + 1],
                        x_d[s, cb * 128:(cb + 1) * 128, r0:r0 + nrows, :])
                    # replicate-pad columns, then the one missing row
                    nc.vector.tensor_copy(
                        x_t[:, cb, rsb0:rsb0 + nrows, 0],
                        x_t[:, cb, rsb0:rsb0 + nrows, 1])
                    nc.vector.tensor_copy(
                        x_t[:, cb, rsb0:rsb0 + nrows, W + 1],
                        x_t[:, cb, rsb0:rsb0 + nrows, W])
                    if hf == 0:
                        nc.vector.tensor_copy(x_t[:, cb, 0, :], x_t[:, cb, 1, :])
                    else:
                        nc.vector.tensor_copy(x_t[:, cb, PADR - 1, :],
                                              x_t[:, cb, PADR - 2, :])
                    # winograd input transform over row pairs:
                    # v0=r0-r2, v1=r1+r2, v2=r2-r1, v3=r1-r3
                    ev0 = x_t[:, cb, 0:2 * NPAIR:2, :]        # rows 0,2..30
                    ev1 = x_t[:, cb, 2:2 * NPAIR + 2:2, :]    # rows 2,4..32
                    od0 = x_t[:, cb, 1:2 * NPAIR + 1:2, :]    # rows 1,3..31
                    od1 = x_t[:, cb, 3:2 * NPAIR + 2:2, :]    # rows 3,5..33
                    nc.vector.tensor_sub(v_t[:, cb, 0], ev0, ev1)
                    nc.vector.tensor_add(v_t[:, cb, 1], od0, ev1)
                    nc.vector.tensor_sub(v_t[:, cb, 2], ev1, od0)
                    nc.vector.tensor_sub(v_t[:, cb, 3], od0, od1)
                if s == 0 and hf == 0:
                    nc.sync.dma_start(sig_sb[:], sig_d[:])

                for ob in range(NOB):
                    for ch in range(NCH):
                        pts = [pspool.tile([128, PPC * W], f32,
                                           name="ps", tag="ps")
                               for _ in range(NPOS)]
                        # cin-block-major accumulation: the first MMs only
                        # need cin-block 0, overlapping later DMAs
                        for cb in range(NCB):
                            for kw in range(KS):
                                for pos in range(NPOS):
                                    lhsT = wt_sb[:, cb, pos, kw,
                                                 ob * 128:(ob + 1) * 128]
                                    rhs = v_t[:, cb, pos,
                                              ch * PPC:(ch + 1) * PPC,
                                              kw:kw + W]
                                    nc.tensor.matmul(
                                        pts[pos][:], lhsT, rhs,
                                        start=(cb == 0 and kw == 0),
                                        stop=(cb == NCB - 1 and kw == KS - 1))
                        # sigma_inv scale on ScalarE (doubles as PSUM evict)
                        sms = []
                        for pos in range(NPOS):
                            sm = smpool.tile([128, PPC * W], f32,
                                             name="sm", tag="sm")
                            nc.scalar.mul(sm[:], pts[pos][:],
                                          sig_sb[:, s, ob:ob + 1])
                            sms.append(sm)
                        # output rows: even = m0+m1+m2, odd = m1-m2-m3
                        te = opool.tile([128, PPC * W], f32, name="te", tag="t")
                        oe = opool.tile([128, PPC * W], f32, name="oe", tag="oo")
                        to = opool.tile([128, PPC * W], f32, name="to", tag="t")
                        oo = opool.tile([128, PPC * W], f32, name="oo", tag="oo")
                        nc.vector.tensor_add(te[:], sms[0][:], sms[1][:])
                        nc.vector.tensor_add(oe[:], te[:], sms[2][:])
                        nc.vector.tensor_sub(to[:], sms[1][:], sms[2][:])
                        nc.vector.tensor_sub(oo[:], to[:], sms[3][:])
                        row0 = hf * HALF + ch * PPC * 2
                        dst = out_d[s, ob * 128:(ob + 1) * 128,
                                    row0 * W:(row0 + 2 * PPC) * W]
                        dst = dst.rearrange("c (p two w) -> c p two w",
                                            two=2, w=W)
                        oe3 = oe.rearrange("c (p w) -> c p w", w=W)
                        oo3 = oo.rearrange("c (p w) -> c p w", w=W)
                        nc.sync.dma_start(dst[:, :, 0, :], oe3[:])
                        nc.sync.dma_start(dst[:, :, 1, :], oo3[:])
    nc.compile()
    _cache["nc"] = nc
    return nc


def _prelu(z, a):
    return np.where(z >= 0, z, a * z)


_G = np.array([[1.0, 0.0, 0.0],
               [0.5, 0.5, 0.5],
               [0.5, -0.5, 0.5],
               [0.0, 0.0, 1.0]], dtype=np.float32)


def _prepare(inputs):
    x = np.asarray(inputs["x"], dtype=np.float32)
    s = np.asarray(inputs["s"], dtype=np.float32)
    map_w0 = np.asarray(inputs["map_w0"], dtype=np.float32)
    map_b0 = np.asarray(inputs["map_b0"], dtype=np.float32)
    a0 = np.asarray(inputs["prelu_a0"], dtype=np.float32)
    map_w1 = np.asarray(inputs["map_w1"], dtype=np.float32)
    map_b1 = np.asarray(inputs["map_b1"], dtype=np.float32)
    a1 = np.asarray(inputs["prelu_a1"], dtype=np.float32)
    style_w = np.asarray(inputs["style_w"], dtype=np.float32)
    style_b = np.asarray(inputs["style_b"], dtype=np.float32)
    conv_w = np.asarray(inputs["conv_w"], dtype=np.float32)

    c_lin = np.float32(1.0 / np.sqrt(DLAT))
    z = _prelu(s @ (map_w0 * c_lin).T + map_b0, a0)
    z = _prelu(z @ (map_w1 * c_lin).T + map_b1, a1)
    style = z @ (style_w * c_lin).T + style_b          # [B, CIN]

    c_conv = 1.0 / np.sqrt(CIN * KS * KS)
    w2 = ((conv_w.astype(np.float64) * c_conv) ** 2).sum(axis=(2, 3))  # [COUT, CIN]
    sig2 = (style.astype(np.float64) ** 2) @ w2.T                      # [B, COUT]
    sig_inv = (1.0 / np.sqrt(sig2 + EPS)).astype(np.float32)
    msc = (style * np.float32(c_conv)).astype(np.float32)              # [B, CIN]

    # per-sample power-of-2 normalizer keeps msc*x in fp16's normal range;
    # undone exactly in the fp32 output scale
    rms = np.sqrt(np.mean((msc.astype(np.float64)) ** 2, axis=1)) + 1e-30
    k = np.clip(np.round(-np.log2(rms)), -20, 40).astype(np.int32)     # [B]
    pw = np.exp2(k.astype(np.float32))                                  # 2^k
    msc_n = msc * pw[:, None]
    sig_n = sig_inv / pw[:, None]

    # fold the per-cin style scale into x on host, cast fp16
    x_scaled = (x * msc_n[:, :, None, None]).astype(np.float16)

    # winograd weight transform over kh: U[pos] = sum_kh G[pos,kh] w[..,kh,..]
    # conv_w: [COUT, CIN, KH, KW] -> U: [COUT, CIN, NPOS, KW]
    u = np.einsum("pk,ockw->ocpw", _G, conv_w).astype(np.float16)
    # -> [NCB, 128, NPOS, KW, COUT]
    wt_host = np.ascontiguousarray(
        u.reshape(COUT, NCB, 128, NPOS, KS).transpose(1, 2, 3, 4, 0))

    sig_r = sig_n.reshape(B, NOB, 128)
    in_maps = []
    for c in range(N_CORES):
        sl = slice(c * SPC, (c + 1) * SPC)
        in_maps.append({
            "x": np.ascontiguousarray(x_scaled[sl]),
            "wt": wt_host,
            "sig": np.ascontiguousarray(sig_r[sl].transpose(2, 0, 1)),
        })
    return in_maps


def run(inputs, **spmd_kwargs):
    nc = _build()
    in_maps = _prepare(inputs)
    res = bass_utils.run_bass_kernel_spmd(
        nc, in_maps, core_ids=list(range(N_CORES)), **spmd_kwargs)
    out = np.concatenate(
        [res.results[c]["out"].reshape(SPC, COUT, H, W)
         for c in range(N_CORES)], axis=0)
    return out, res


def kernel(**inputs) -> np.ndarray:
    out, _ = run(inputs)
    return out


# revision 12
# speedup vs baseline: 1.6822x; 1.0547x over previous
"""Trainium2 Bass kernel for Conv2dWeightModulate (StyleGAN2-style modulated conv).

Math restructure 1 (modulation): the per-sample modulated conv
    out[b] = conv(conv_w * c * style[b,cin] * sigma_inv[b,cout], x_pad[b])
is rewritten as
    out[b,cout] = sigma_inv[b,cout] * conv(conv_w, (x[b] * c*style[b,cin])_pad)
so the conv weights are sample-independent (resident in SBUF) and the
per-sample modulation becomes a per-input-channel scale of x plus a
per-output-channel scale of the result. sigma has the closed form
    sigma^2[b,cout] = c^2 * sum_cin style[b,cin]^2 * sum_k conv_w[cout,cin,k]^2
computed on host (tiny [B,CIN] x [CIN,COUT] product), as is the 3-layer
mapping network producing style (all [16,512]-sized, <0.01% of FLOPs).

Math restructure 2 (Winograd F(2,3) along the height axis): each pair of
output rows (2p, 2p+1) is computed from 4 transformed input rows
    v0 = d[2p]-d[2p+2], v1 = d[2p+1]+d[2p+2],
    v2 = d[2p+2]-d[2p+1], v3 = d[2p+1]-d[2p+3]
with host-transformed weights U = G @ w over the kh axis
    (G = [[1,0,0],[.5,.5,.5],[.5,-.5,.5],[0,0,1]])
and output rows  out[2p] = M0+M1+M2,  out[2p+1] = M1-M2-M3  where
    M[pos] = sum_{cin,kw} U[pos,kw] * v[pos]  (shifted by kw).
This cuts tensor-engine MACs by 1.5x (12 accumulation steps per output
tile instead of 18 row-equivalents); the width axis stays direct (3 taps
against a replicate-padded 66-wide SBUF image).

Device: data-parallel over batch, 2 samples per core on 8 cores, fp16
operands (PE at 1 col/cycle, fp32 PSUM accumulate). Per (sample, half,
cout-block, row-chunk): 4 PSUM banks hold M[0..3] for 8 row-pairs x 64
cols; 48 accumulating 128x128 @ 128x512 matmuls fill them; ScalarE applies
sigma_inv during PSUM->SBUF eviction and VectorE forms the two output-row
combinations in fp32. A per-sample power-of-2 prescale keeps scaled x in
fp16's normal range and is undone exactly in the fp32 output scale.
"""

import numpy as np
from contextlib import ExitStack

import concourse.bass as bass
import concourse.tile as tile
from concourse import bacc, mybir
from concourse import bass_utils

B, CIN, COUT, KS, H, W, DLAT = 16, 512, 512, 3, 64, 64, 512
EPS = 1e-8
N_CORES = 8
SPC = B // N_CORES          # samples per core
NCB = CIN // 128            # cin blocks
NOB = COUT // 128           # cout blocks
HALF = H // 2               # rows per half-image
NPAIR = HALF // 2           # winograd row-pairs per half (16)
PPC = 8                     # row-pairs per PSUM chunk (8 pairs * 64 = 512)
NCH = NPAIR // PPC          # chunks per half (2)
PADW = W + 2
PADR = HALF + 2
NPOS = 4                    # winograd positions
_cache = {}


def _build():
    if "nc" in _cache:
        return _cache["nc"]
    f32 = mybir.dt.float32
    f16 = mybir.dt.float16
    nc = bacc.Bacc("TRN2", target_bir_lowering=False, debug=False,
                   num_devices=N_CORES)
    x_d = nc.dram_tensor("x", [SPC, CIN, H, W], f16, kind="ExternalInput").ap()
    # U[cb, p, pos, kw, cout]
    wt_d = nc.dram_tensor("wt", [NCB, 128, NPOS, KS, COUT], f16,
                          kind="ExternalInput").ap()
    sig_d = nc.dram_tensor("sig", [128, SPC, NOB], f32,
                           kind="ExternalInput").ap()
    out_d = nc.dram_tensor("out", [SPC, COUT, H * W], f32,
                           kind="ExternalOutput").ap()

    with tile.TileContext(nc) as tc, ExitStack() as ctx:
        cpool = ctx.enter_context(tc.tile_pool(name="const", bufs=1))
        stpool = ctx.enter_context(tc.tile_pool(name="stage", bufs=4))
        vpool = ctx.enter_context(tc.tile_pool(name="v", bufs=2))
        smpool = ctx.enter_context(tc.tile_pool(name="sm", bufs=8))
        opool = ctx.enter_context(tc.tile_pool(name="o", bufs=8))
        pspool = ctx.enter_context(tc.tile_pool(name="ps", bufs=8, space="PSUM"))

        wt_sb = cpool.tile([128, NCB, NPOS, KS, COUT], f16)
        sig_sb = cpool.tile([128, SPC, NOB], f32)

        nrows = HALF + 1  # 33 original rows cover one half
        for s in range(SPC):
            for hf in range(2):
                # V is built straight from a contiguous staging copy of the
                # original rows; replicate-padding commutes with the (linear)
                # row transform, so V's column pads are plain copies and the
                # one clamped boundary row becomes a single-pair fixup.
                v_t = vpool.tile([128, NCB, NPOS, NPAIR, PADW], f16)
                r0 = 0 if hf == 0 else HALF - 1
                for cb in range(NCB):
                    st = stpool.tile([128, nrows * W], f16, name="st", tag="st")
                    nc.sync.dma_start(
                        st[:],
                        x_d[s, cb * 128:(cb + 1) * 128,
                            r0:r0 + nrows, :].rearrange("c a b -> c (a b)"))
                    if s == 0 and hf == 0:
                        # weight loads interleaved after each x block so
                        # cin-block cb's matmuls start as soon as possible
                        nc.sync.dma_start(wt_sb[:, cb], wt_d[cb])
                    sr = st.rearrange("c (a b) -> c a b", b=W)
                    # winograd input transform over row pairs (d = padded
                    # rows): v0=d0-d2, v1=d1+d2, v2=d2-d1, v3=d1-d3
                    vi = v_t[:, cb]
                    if hf == 0:
                        # d_i = sr[i-1] for i>=1, d_0 = sr[0] (clamped row)
                        m = slice(1, NPAIR)      # pairs 1..15 regular
                        nc.vector.tensor_sub(vi[:, 0, m, 1:W + 1],
                                             sr[:, 1:31:2], sr[:, 3:33:2])
                        nc.vector.tensor_add(vi[:, 1, m, 1:W + 1],
                                             sr[:, 2:32:2], sr[:, 3:33:2])
                        nc.vector.tensor_sub(vi[:, 2, m, 1:W + 1],
                                             sr[:, 3:33:2], sr[:, 2:32:2])
                        nc.vector.tensor_sub(vi[:, 3, m, 1:W + 1],
                                             sr[:, 2:32:2], sr[:, 4:33:2])
                        # pair 0 fixup: d0=d1=sr0, d2=sr1, d3=sr2
                        nc.vector.tensor_sub(vi[:, 0, 0:1, 1:W + 1],
                                             sr[:, 0:1], sr[:, 1:2])
                        nc.vector.tensor_add(vi[:, 1, 0:1, 1:W + 1],
                                             sr[:, 0:1], sr[:, 1:2])
                        nc.vector.tensor_sub(vi[:, 2, 0:1, 1:W + 1],
                                             sr[:, 1:2], sr[:, 0:1])
                        nc.vector.tensor_sub(vi[:, 3, 0:1, 1:W + 1],
                                             sr[:, 0:1], sr[:, 2:3])
                    else:
                        # d_i = sr[i] for i<=32, d_33 = sr[32] (clamped row)
                        nc.vector.tensor_sub(vi[:, 0, :, 1:W + 1],
                                             sr[:, 0:32:2], sr[:, 2:33:2])
                        nc.vector.tensor_add(vi[:, 1, :, 1:W + 1],
                                             sr[:, 1:33:2], sr[:, 2:33:2])
                        nc.vector.tensor_sub(vi[:, 2, :, 1:W + 1],
                                             sr[:, 2:33:2], sr[:, 1:33:2])
                        m = slice(0, NPAIR - 1)  # pairs 0..14 regular
                        nc.vector.tensor_sub(vi[:, 3, m, 1:W + 1],
                                             sr[:, 1:31:2], sr[:, 3:33:2])
                        # pair 15 fixup: d31=sr31, d33=sr32 (clamped)
                        nc.vector.tensor_sub(vi[:, 3, NPAIR - 1:NPAIR, 1:W + 1],
                                             sr[:, 31:32], sr[:, 32:33])
                    # V column pads (replicate: padded col0==col1, 65==64)
                    nc.vector.tensor_copy(vi[:, :, :, 0], vi[:, :, :, 1])
                    nc.vector.tensor_copy(vi[:, :, :, W + 1], vi[:, :, :, W])
                if s == 0 and hf == 0:
                    nc.sync.dma_start(sig_sb[:], sig_d[:])

                for ob in range(NOB):
                    for ch in range(NCH):
                        pts = [pspool.tile([128, PPC * W], f32,
                                           name="ps", tag="ps")
                               for _ in range(NPOS)]
                        # pos pair (0,1) accumulates fully before (2,3) so
                        # their evictions overlap the second half's matmuls;
                        # cin-block-major so the first MMs only need block 0
                        for pp in (0, 2):
                            for cb in range(NCB):
                                for kw in range(KS):
                                    for pos in (pp, pp + 1):
                                        lhsT = wt_sb[:, cb, pos, kw,
                                                     ob * 128:(ob + 1) * 128]
                                        rhs = v_t[:, cb, pos,
                                                  ch * PPC:(ch + 1) * PPC,
                                                  kw:kw + W]
                                        nc.tensor.matmul(
                                            pts[pos][:], lhsT, rhs,
                                            start=(cb == 0 and kw == 0),
                                            stop=(cb == NCB - 1 and kw == KS - 1))
                        # sigma_inv scale on ScalarE (doubles as PSUM evict)
                        sms = []
                        for pos in range(NPOS):
                            sm = smpool.tile([128, PPC * W], f32,
                                             name="sm", tag="sm")
                            nc.scalar.mul(sm[:], pts[pos][:],
                                          sig_sb[:, s, ob:ob + 1])
                            sms.append(sm)
                        # output rows: even = m0+m1+m2, odd = m1-m2-m3
                        te = opool.tile([128, PPC * W], f32, name="te", tag="t")
                        oe = opool.tile([128, PPC * W], f32, name="oe", tag="oo")
                        to = opool.tile([128, PPC * W], f32, name="to", tag="t")
                        oo = opool.tile([128, PPC * W], f32, name="oo", tag="oo")
                        nc.vector.tensor_add(te[:], sms[0][:], sms[1][:])
                        nc.vector.tensor_add(oe[:], te[:], sms[2][:])
                        nc.vector.tensor_sub(to[:], sms[1][:], sms[2][:])
                        nc.vector.tensor_sub(oo[:], to[:], sms[3][:])
                        row0 = hf * HALF + ch * PPC * 2
                        dst = out_d[s, ob * 128:(ob + 1) * 128,
                                    row0 * W:(row0 + 2 * PPC) * W]
                        dst = dst.rearrange("c (p two w) -> c p two w",
                                            two=2, w=W)
                        oe3 = oe.rearrange("c (p w) -> c p w", w=W)
                        oo3 = oo.rearrange("c (p w) -> c p w", w=W)
                        nc.sync.dma_start(dst[:, :, 0, :], oe3[:])
                        nc.sync.dma_start(dst[:, :, 1, :], oo3[:])
    nc.compile()
    _cache["nc"] = nc
    return nc


def _prelu(z, a):
    return np.where(z >= 0, z, a * z)


_G = np.array([[1.0, 0.0, 0.0],
               [0.5, 0.5, 0.5],
               [0.5, -0.5, 0.5],
               [0.0, 0.0, 1.0]], dtype=np.float32)


def _prepare(inputs):
    x = np.asarray(inputs["x"], dtype=np.float32)
    s = np.asarray(inputs["s"], dtype=np.float32)
    map_w0 = np.asarray(inputs["map_w0"], dtype=np.float32)
    map_b0 = np.asarray(inputs["map_b0"], dtype=np.float32)
    a0 = np.asarray(inputs["prelu_a0"], dtype=np.float32)
    map_w1 = np.asarray(inputs["map_w1"], dtype=np.float32)
    map_b1 = np.asarray(inputs["map_b1"], dtype=np.float32)
    a1 = np.asarray(inputs["prelu_a1"], dtype=np.float32)
    style_w = np.asarray(inputs["style_w"], dtype=np.float32)
    style_b = np.asarray(inputs["style_b"], dtype=np.float32)
    conv_w = np.asarray(inputs["conv_w"], dtype=np.float32)

    c_lin = np.float32(1.0 / np.sqrt(DLAT))
    z = _prelu(s @ (map_w0 * c_lin).T + map_b0, a0)
    z = _prelu(z @ (map_w1 * c_lin).T + map_b1, a1)
    style = z @ (style_w * c_lin).T + style_b          # [B, CIN]

    c_conv = 1.0 / np.sqrt(CIN * KS * KS)
    w2 = ((conv_w.astype(np.float64) * c_conv) ** 2).sum(axis=(2, 3))  # [COUT, CIN]
    sig2 = (style.astype(np.float64) ** 2) @ w2.T                      # [B, COUT]
    sig_inv = (1.0 / np.sqrt(sig2 + EPS)).astype(np.float32)
    msc = (style * np.float32(c_conv)).astype(np.float32)              # [B, CIN]

    # per-sample power-of-2 normalizer keeps msc*x in fp16's normal range;
    # undone exactly in the fp32 output scale
    rms = np.sqrt(np.mean((msc.astype(np.float64)) ** 2, axis=1)) + 1e-30
    k = np.clip(np.round(-np.log2(rms)), -20, 40).astype(np.int32)     # [B]
    pw = np.exp2(k.astype(np.float32))                                  # 2^k
    msc_n = msc * pw[:, None]
    sig_n = sig_inv / pw[:, None]

    # fold the per-cin style scale into x on host, cast fp16
    x_scaled = (x * msc_n[:, :, None, None]).astype(np.float16)

    # winograd weight transform over kh: U[pos] = sum_kh G[pos,kh] w[..,kh,..]
    # conv_w: [COUT, CIN, KH, KW] -> U: [COUT, CIN, NPOS, KW]
    u = np.einsum("pk,ockw->ocpw", _G, conv_w).astype(np.float16)
    # -> [NCB, 128, NPOS, KW, COUT]
    wt_host = np.ascontiguousarray(
        u.reshape(COUT, NCB, 128, NPOS, KS).transpose(1, 2, 3, 4, 0))

    sig_r = sig_n.reshape(B, NOB, 128)
    in_maps = []
    for c in range(N_CORES):
        sl = slice(c * SPC, (c + 1) * SPC)
        in_maps.append({
            "x": np.ascontiguousarray(x_scaled[sl]),
            "wt": wt_host,
            "sig": np.ascontiguousarray(sig_r[sl].transpose(2, 0, 1)),
        })
    return in_maps


def run(inputs, **spmd_kwargs):
    nc = _build()
    in_maps = _prepare(inputs)
    res = bass_utils.run_bass_kernel_spmd(
        nc, in_maps, core_ids=list(range(N_CORES)), **spmd_kwargs)
    out = np.concatenate(
        [res.results[c]["out"].reshape(SPC, COUT, H, W)
         for c in range(N_CORES)], axis=0)
    return out, res


def kernel(**inputs) -> np.ndarray:
    out, _ = run(inputs)
    return out
